# revision 22
# baseline (speedup 1.0000x reference)
"""Adaptive frequency reassemble kernel for 8 TRN2 NeuronCores.

Sharding: pure data parallel over (B, D): core i owns batch b=i//4 and
d-slab [8*(i%4), 8*(i%4)+8) -> 32768 positions/core.  x_lf / x_hf are
stacked into one [128, 32768] tensor per core (lf channels on partitions
0-63, hf on 64-127) and fed to the device in bf16, halving the input
HBM traffic vs f32.  The output leaves the device in bf16 on all 128
partitions ([128, NPOS/2] with channels x position-parity packing),
quartering the output DMA per-partition byte count vs the f32 [64, NPOS]
layout.  Host casts both ways (cheap, not on the device critical path).

Algebraic folds (host, exact):
  tok_t  = tokens @ W_t2f.T + b_t2f
  M      = (tok_t @ W_delta.T) * scale
  G      = M @ W_gate.T                  [8, 64]
  bg2    = W_gate @ (b_delta*scale) + b_gate
so   gate_pre = G.T @ softmax_weights + bg2, and the final combine uses
  out = base*(1+sigmoid(gate_pre)) = (base/2) * (3 + tanh(gate_pre/2)),
which keeps every ACT op (Exp, Tanh, Relu) inside the single
`exp_and_others` activation table set: one LoadActFuncSet for the whole
kernel.  Likewise the SE-gate MLP sigmoids become 0.5+0.5*tanh(z/2).

Attention sub-tiles are packed 4x at PE quadrant offsets {0,32,64,96},
so exp/softmax-denominator/normalize/gate-matmul all run on 512-wide
free dims covering 2048 positions at a time.

Per-channel context sums (for the SE gate) ride tensor_scalar copies
with accum_out, split DVE/GPSIMD so no engine exceeds the PE wall;
the tiny [128] AllReduce across the 4 cores sharing each batch is
issued in two staggered halves so its latency hides under phase A.
"""

import sys

import numpy as np

if "/opt/trn_rl_repo" not in sys.path:
    sys.path.insert(0, "/opt/trn_rl_repo")

_B, _C, _D, _H, _W = 2, 64, 32, 64, 64
_K = 8
_NCORES = 8
_NPOS = (_B * _D // _NCORES) * _H * _W  # 32768 positions per core
_NT = 1024   # DMA tile width
_NS = 512    # attention sub-tile width (pack factor 4)
_NPAIR = _NPOS // (2 * _NT)  # 16 pairs of tiles (2048 positions each)

# phase-B combine: pair p -> direct-from-PSUM DVE path if p % _BCOMB_MOD == 0,
# else ACT copies psum->bf16 first so the DVE combine runs in 16-bit fast mode
_BCOMB_MOD = 3

_NC_CACHE = {}


def _build_nc(repeat=1, no_cc=False):
    import concourse.bass as bass  # noqa: F401
    import concourse.bacc as bacc
    import concourse.mybir as mybir
    from concourse import tile
    from concourse.alu_op_type import AluOpType

    f32 = mybir.dt.float32
    bf16 = mybir.dt.bfloat16
    AF = mybir.ActivationFunctionType

    nc = bacc.Bacc(None, num_devices=1 if no_cc else _NCORES)

    xs_d = nc.declare_dram_parameter("xs", [128, _NPOS], bf16, isOutput=False)
    # params packed into two blocks: one DMA + one bf16 convert at start
    pbf_d = nc.declare_dram_parameter("pbf", [128, 456], bf16, isOutput=False)
    pf_d = nc.declare_dram_parameter("pf32", [128, 209], f32, isOutput=False)
    out_d = nc.declare_dram_parameter("out", [128, _NPOS // 2], bf16,
                                      isOutput=True)

    cc_in = nc.dram_tensor("cc_in", [128, 1], f32)
    cc_out = nc.dram_tensor("cc_out", [128, 1], f32)
    cc_in2 = nc.dram_tensor("cc_in2", [128, 1], f32)
    cc_out2 = nc.dram_tensor("cc_out2", [128, 1], f32)

    ntiles = _NPOS // _NT  # 32

    rep_range = range(repeat)
    with tile.TileContext(nc) as tc:
        with (
            tc.tile_pool(name="const", bufs=1) as cpool,
            tc.tile_pool(name="res", bufs=1) as rpool,
            tc.tile_pool(name="work", bufs=4) as wpool,
        ):
            # param loads ride the idle ACT sequencer so the SP queue
            # head belongs to the input stream from cycle zero
            pbf16 = cpool.tile([128, 456], bf16)
            nc.scalar.dma_start(pbf16[:], pbf_d[:])
            pf_s = cpool.tile([128, 209], f32)
            nc.scalar.dma_start(pf_s[:], pf_d[:])
            # views into the packed blocks
            tokbf = pbf16[:, 0:32]
            b36bf = pbf16[:, 32:68]
            b4bf = pbf16[:, 68:72]
            bt36bf = pbf16[0:36, 72:200]
            g4abf = pbf16[:, 200:328]
            g4bbf = pbf16[:, 328:456]
            wst_s = pf_s[:, 0:16]
            wglf_s = pf_s[0:16, 16:80]
            wghf_s = pf_s[0:16, 80:144]
            i1_s = pf_s[:, 144:208]
            bgh_s = pf_s[:, 208:209]

            for _rep in rep_range:
                sxbf = rpool.tile([128, _NPOS], bf16)       # 64 KB/part
                gat = rpool.tile([128, _NPOS // 2], bf16)   # 32 KB/part
                rs_cols = rpool.tile([128, _NPAIR], f32)    # per-pair row sums
                junkD = rpool.tile([128, 2 * _NT], bf16)

                # ---- Phase A: stream x, attention weights + context sums ----
                # Software-pipelined over quads (2 pairs = 4096 positions):
                # iteration `it` emits, per engine stream,
                #   PE : scores_it | psRB_{it-1} | denom_it | psG_{it-2}
                #   ACT: exp_it | tanh_{it-2}
                #   DVE: rowsums_it | En_{it-1} | recip_it
                # so every op's cross-engine inputs are a full iteration old
                # and no engine waits mid-stream.
                NQ = _NPAIR // 2
                EQ = [None] * NQ     # per-quad [E_p0, E_p1]
                RQ = [None] * NQ     # per-quad R36
                ENQ = [None] * NQ    # per-quad [En_p0, En_p1]
                psa_ctx = tc.tile_pool(name="psA", bufs=1, space="PSUM")
                with psa_ctx as psa:
                    for it in range(NQ + 2):
                        if it < NQ:
                            q = it
                            Es = []
                            for m in range(2):
                                p = 2 * q + m
                                for h in range(2):
                                    t = 2 * p + h
                                    sl = slice(t * _NT, (t + 1) * _NT)
                                    nc.sync.dma_start(sxbf[:, sl],
                                                      xs_d[:, sl])
                                # context row-sum rides a bf16 copy with
                                # accum_out (fast DVE mode; Pool rejects the
                                # accum opcode in walrus codegen); one op per
                                # 2048-position pair amortizes the overhead
                                nc.vector.tensor_scalar(
                                    junkD[:], sxbf[:, 2 * p * _NT:
                                                   2 * (p + 1) * _NT],
                                    1.0, 0.0,
                                    AluOpType.mult, AluOpType.add,
                                    accum_out=rs_cols[:, p:p + 1],
                                )
                                psS = psa.tile([128, _NS], f32, tag="ps2k",
                                               name="psS", bufs=4)
                                for j in range(4):
                                    nc.tensor.matmul(
                                        psS[32 * j:32 * (j + 1), :], tokbf,
                                        sxbf[:, (4 * p + j) * _NS:
                                             (4 * p + j + 1) * _NS],
                                        start=True, stop=True,
                                        tile_position=(0, 32 * j),
                                    )
                                E = wpool.tile([128, _NS], bf16, tag="E",
                                               bufs=6)
                                nc.scalar.activation(E[:], psS[:], AF.Exp)
                                Es.append(E)
                            EQ[q] = Es
                        # PE: recip broadcast for quad it-1
                        q1 = it - 1
                        psRBs = []
                        if 0 <= q1 < NQ:
                            for m in range(2):
                                psRB = psa.tile([128, _NS], f32,
                                                tag="ps2k", name="psRB",
                                                bufs=4)
                                nc.tensor.matmul(
                                    psRB[:], bt36bf[32 * m:32 * m + 4, :],
                                    RQ[q1][32 * m:32 * m + 4, :],
                                    start=True, stop=True,
                                    tile_position=(32 * m, 0),
                                )
                                psRBs.append(psRB)
                        # PE: softmax denominators for quad it
                        psD = None
                        if it < NQ:
                            psD = psa.tile([36, _NS], f32, tag="ps2k",
                                           name="psD", bufs=4)
                            # first mm writes all 36 rows so the [36,512]
                            # recip never reads uninitialized PSUM
                            nc.tensor.matmul(psD[:], b36bf, EQ[it][0][:],
                                             start=True, stop=True,
                                             tile_position=(0, 0))
                            nc.tensor.matmul(psD[32:36, :], b4bf,
                                             EQ[it][1][:],
                                             start=True, stop=True,
                                             tile_position=(0, 32),
                                             skip_group_check=True)
                        # DVE: normalized attention for quad it-1
                        if 0 <= q1 < NQ:
                            Ens = []
                            for m in range(2):
                                En = wpool.tile([128, _NS], bf16, tag="En",
                                                bufs=6)
                                nc.vector.tensor_tensor(
                                    out=En[:], in0=EQ[q1][m][:],
                                    in1=psRBs[m][:], op=AluOpType.mult,
                                )
                                Ens.append(En)
                            ENQ[q1] = Ens
                            EQ[q1] = None
                        # DVE: reciprocal for quad it
                        if it < NQ:
                            R36 = wpool.tile([36, _NS], bf16, tag="R36",
                                             bufs=3)
                            with nc.allow_low_precision(
                                    "softmax recip in bf16: ~0.4% on attn "
                                    "weights, far under the 2e-2 gate"):
                                nc.vector.reciprocal(R36[:], psD[:])
                            RQ[it] = R36
                        # PE: gate matmuls + ACT: tanh cache for quad it-2
                        q2 = it - 2
                        if 0 <= q2 < NQ:
                            for m in range(2):
                                p = 2 * q2 + m
                                psG = psa.tile([128, 2 * _NS], f32,
                                               tag="psG", name="psG",
                                               bufs=2)
                                nc.tensor.matmul(psG[:, 0:_NS], g4abf,
                                                 ENQ[q2][m][:],
                                                 start=True, stop=True)
                                nc.tensor.matmul(psG[:, _NS:2 * _NS],
                                                 g4bbf, ENQ[q2][m][:],
                                                 start=True, stop=True)
                                # gate cache: tanh((psG + bg2)/2); sigmoid
                                # folds into phase B as (3 + tanh)/2
                                nc.scalar.activation(
                                    gat[:, _NT * p:_NT * (p + 1)], psG[:],
                                    AF.Tanh, bias=bgh_s, scale=0.5,
                                )
                                if p % 2 == 0:
                                    # in-place g3 = 3 + tanh (on the idle
                                    # GPSIMD engine) so this pair's phase-B
                                    # combine is a fast 16-bit mult
                                    with nc.allow_low_precision(
                                            "bf16 gate cache, ~0.3%"):
                                        nc.gpsimd.tensor_scalar(
                                            gat[:, _NT * p:_NT * (p + 1)],
                                            gat[:, _NT * p:_NT * (p + 1)],
                                            1.0, 3.0,
                                            AluOpType.mult, AluOpType.add,
                                        )
                            ENQ[q2] = None
                        if it == NQ // 2 - 1 and not no_cc:
                            # first-half context partial: its AllReduce
                            # latency hides under the second half of phase A
                            rs_a = rpool.tile([128, 1], f32)
                            nc.vector.tensor_reduce(
                                rs_a[:], rs_cols[:, 0:_NPAIR // 2],
                                axis=mybir.AxisListType.X, op=AluOpType.add,
                            )
                            nc.sync.dma_start(cc_in[:], rs_a[:])
                            nc.gpsimd.collective_compute(
                                "AllReduce", AluOpType.add,
                                replica_groups=[[0, 1, 2, 3], [4, 5, 6, 7]],
                                ins=[cc_in[:]], outs=[cc_out[:]],
                            )
                            cc_sb = rpool.tile([128, 2], f32)
                            nc.sync.dma_start(cc_sb[:, 0:1], cc_out[:])

                    # ---- second-half context AllReduce + gate MLP ----
                    rs = rpool.tile([128, 1], f32)
                    nc.vector.tensor_reduce(
                        rs[:], rs_cols[:, _NPAIR // 2:_NPAIR],
                        axis=mybir.AxisListType.X, op=AluOpType.add,
                    )
                    if no_cc:
                        ctxs = rs
                    else:
                        nc.sync.dma_start(cc_in2[:], rs[:])
                        nc.gpsimd.collective_compute(
                            "AllReduce", AluOpType.add,
                            replica_groups=[[0, 1, 2, 3], [4, 5, 6, 7]],
                            ins=[cc_in2[:]], outs=[cc_out2[:]],
                        )
                        nc.sync.dma_start(cc_sb[:, 1:2], cc_out2[:])
                        ctxs = rpool.tile([128, 1], f32)
                        nc.vector.tensor_reduce(
                            ctxs[:], cc_sb[:], axis=mybir.AxisListType.X,
                            op=AluOpType.add,
                        )
                    ps1 = psa.tile([16, 1], f32, tag="ps2k", name="ps1", bufs=4)
                    nc.tensor.matmul(ps1[:], wst_s, ctxs[:],
                                     start=True, stop=True)
                    sh = rpool.tile([16, 1], f32)
                    nc.scalar.activation(sh[:], ps1[:], AF.Relu)
                    ps2 = psa.tile([64, 1], f32, tag="ps2k", name="ps2", bufs=4)
                    nc.tensor.matmul(ps2[:], wglf_s, sh[:],
                                     start=True, stop=True)
                    ps3 = psa.tile([64, 1], f32, tag="ps2k", name="ps3", bufs=4)
                    nc.tensor.matmul(ps3[:], wghf_s, sh[:],
                                     start=True, stop=True)
                    # w = 2*sigmoid(z); base/2 uses sigmoid(z) = .5+.5*tanh(z/2)
                    wvt = rpool.tile([128, 1], f32)
                    nc.scalar.activation(wvt[0:64, :], ps2[:], AF.Tanh,
                                         scale=0.5)
                    nc.scalar.activation(wvt[64:128, :], ps3[:], AF.Tanh,
                                         scale=0.5)
                    # wsel = I1h*(1+t) = I1*(0.5+0.5*t) = I1*sigmoid(z)
                    wsel = rpool.tile([128, 64], bf16)
                    nc.vector.scalar_tensor_tensor(
                        wsel[:], i1_s, wvt[:, 0:1], i1_s,
                        AluOpType.mult, AluOpType.add,
                    )
                # ---- Phase B: base matmul + combine, stream out ----
                with (
                    tc.tile_pool(name="psB", bufs=4, space="PSUM") as psbp,
                    tc.tile_pool(name="outp", bufs=6) as opool,
                ):
                    for p in range(_NPAIR):
                        psB = psbp.tile([128, 2 * _NS], f32, tag="psB")
                        for b in range(2):
                            for beta in range(2):
                                v = 4 * p + 2 * b + beta
                                nc.tensor.matmul(
                                    psB[64 * beta:64 * beta + 64,
                                        b * _NS:(b + 1) * _NS],
                                    wsel[:], sxbf[:, v * _NS:(v + 1) * _NS],
                                    start=True, stop=True,
                                )
                        outt = opool.tile([128, _NT], bf16, tag="outt")
                        gsl = gat[:, _NT * p:_NT * (p + 1)]
                        with nc.allow_low_precision(
                                "bf16 output: ~0.2% rounding, far under "
                                "the 2e-2 gate"):
                            if p % 2 == 0:
                                # g3 cached: ACT copies psum->bf16, then the
                                # combine is a 16-bit tensor_tensor mult
                                pb = opool.tile([128, _NT], bf16, tag="pb")
                                nc.scalar.activation(pb[:], psB[:], AF.Copy)
                                eng = nc.gpsimd if p in (0, 2, 4) else nc.vector
                                eng.tensor_tensor(
                                    out=outt[:], in0=gsl, in1=pb[:],
                                    op=AluOpType.mult,
                                )
                            else:
                                nc.vector.scalar_tensor_tensor(
                                    outt[:], gsl, 3.0, psB[:],
                                    AluOpType.add, AluOpType.mult,
                                )
                        nc.sync.dma_start(
                            out_d[:, _NT * p:_NT * (p + 1)],
                            outt[:],
                        )

    nc.compile()
    nc.finalize()
    return nc


def _get_nc(repeat=1, no_cc=False):
    key = f"nc{repeat}_{no_cc}"
    if key not in _NC_CACHE:
        _NC_CACHE[key] = _build_nc(repeat, no_cc)
    return _NC_CACHE[key]


def _host_params(inputs):
    f = np.float32
    tokens = np.asarray(inputs["tokens"], f)
    scale = float(np.asarray(inputs["scale"]).reshape(-1)[0])
    sf = _C ** -0.5
    tok32 = np.zeros((128, 32), f)
    tok32[0:64, 0:_K] = tokens.T * sf
    tok32[64:128, 0:_K] = tokens.T * sf
    tok_t = tokens @ np.asarray(inputs["W_t2f"], f).T + np.asarray(
        inputs["b_t2f"], f)
    M = (tok_t @ np.asarray(inputs["W_delta"], f).T) * scale
    W_gate = np.asarray(inputs["W_gate"], f)
    G = M @ W_gate.T  # [8, 64]
    bg2v = (W_gate @ (np.asarray(inputs["b_delta"], f) * scale)
            + np.asarray(inputs["b_gate"], f))
    # tanh bias: (psG + bg2)/2, stacked for both row halves
    bgh = 0.5 * np.concatenate([bg2v, bg2v])[:, None]  # [128,1]
    # quadrant-packed selector / replication matrices (bands at 32j)
    B4 = np.zeros((128, 4), f)
    B36 = np.zeros((128, 36), f)
    Bt36 = np.zeros((36, 128), f)
    G4a = np.zeros((128, 128), f)
    G4b = np.zeros((128, 128), f)
    for j in range(4):
        B4[32 * j:32 * j + 8, j] = 1.0
        B36[32 * j:32 * j + 8, j] = 1.0
    # rows 4-35 of the pair-0 denominator tile get E[0,:] (finite, unused)
    B36[0, 4:36] = 1.0
    for m in range(2):
        for j in range(4):
            Bt36[32 * m + j, 32 * j:32 * j + 8] = 1.0
    for k in range(_K):
        G4a[k, 0:64] = G[k]
        G4a[32 + k, 64:128] = G[k]
        G4b[64 + k, 0:64] = G[k]
        G4b[96 + k, 64:128] = G[k]
    WsT = np.ascontiguousarray(
        np.asarray(inputs["W_shared"], f).T / (_D * _H * _W))
    WglfT = np.ascontiguousarray(np.asarray(inputs["W_glf"], f).T)
    WghfT = np.ascontiguousarray(np.asarray(inputs["W_ghf"], f).T)
    eye1 = 0.5 * np.eye(64, dtype=f)
    I1 = np.ascontiguousarray(np.concatenate([eye1, eye1], 0))
    pbf = np.zeros((128, 456), f)
    pbf[:, 0:32] = tok32
    pbf[:, 32:68] = B36
    pbf[:, 68:72] = B4
    pbf[0:36, 72:200] = Bt36
    pbf[:, 200:328] = G4a
    pbf[:, 328:456] = G4b
    pf32 = np.zeros((128, 209), f)
    pf32[:, 0:16] = WsT
    pf32[0:16, 16:80] = WglfT
    pf32[0:16, 80:144] = WghfT
    pf32[:, 144:208] = I1
    pf32[:, 208:209] = bgh
    return {"pbf": pbf, "pf32": pf32}


def _make_in_maps(inputs):
    import ml_dtypes

    x_hf = np.asarray(inputs["x_hf"], np.float32)
    x_lf = np.asarray(inputs["x_lf"], np.float32)
    params = _host_params(inputs)
    in_maps = []
    for i in range(_NCORES):
        b, d0 = i // 4, 8 * (i % 4)
        xl = x_lf[b, :, d0:d0 + 8].reshape(64, -1)
        xh = x_hf[b, :, d0:d0 + 8].reshape(64, -1)
        xs = np.ascontiguousarray(
            np.concatenate([xl, xh], 0)).astype(ml_dtypes.bfloat16)
        m = {"xs": xs}
        m.update(params)
        in_maps.append(m)
    return in_maps


def _decode_out(raw):
    # raw [128, NPOS/2] bf16: row 64*beta+c, col 512*(2p+b)+f
    #  -> channel c, position 512*(4p+2b+beta)+f
    o = np.asarray(raw).astype(np.float32)
    o = o.reshape(2, 64, _NPAIR, 2, _NS)          # [beta, c, p, b, f]
    o = o.transpose(1, 2, 3, 0, 4)                # [c, p, b, beta, f]
    return o.reshape(64, _NPOS)


def kernel(**inputs):
    from concourse.bass_utils import run_bass_kernel_spmd

    in_maps = _make_in_maps(inputs)
    nc = _get_nc()
    res = run_bass_kernel_spmd(nc, in_maps, list(range(_NCORES)))
    out = np.empty((_B, _C, _D, _H, _W), np.float32)
    for i in range(_NCORES):
        b, d0 = i // 4, 8 * (i % 4)
        out[b, :, d0:d0 + 8] = _decode_out(res.results[i]["out"]).reshape(
            64, 8, _H, _W)
    return out


# revision 25
# speedup vs baseline: 5.4215x; 5.4215x over previous
"""Adaptive frequency reassemble kernel for 8 TRN2 NeuronCores.

Sharding: pure data parallel over (B, D): core i owns batch b=i//4 and
d-slab [8*(i%4), 8*(i%4)+8) -> 32768 positions/core.  x_lf / x_hf are
stacked into one [128, 32768] tensor per core (lf channels on partitions
0-63, hf on 64-127) and fed to the device in bf16, halving the input
HBM traffic vs f32.  The output leaves the device in bf16 on all 128
partitions ([128, NPOS/2] with channels x position-parity packing),
quartering the output DMA per-partition byte count vs the f32 [64, NPOS]
layout.  Host casts both ways (cheap, not on the device critical path).

Algebraic folds (host, exact):
  tok_t  = tokens @ W_t2f.T + b_t2f
  M      = (tok_t @ W_delta.T) * scale
  G      = M @ W_gate.T                  [8, 64]
  bg2    = W_gate @ (b_delta*scale) + b_gate
so   gate_pre = G.T @ softmax_weights + bg2, and the final combine uses
  out = base*(1+sigmoid(gate_pre)) = (base/2) * (3 + tanh(gate_pre/2)),
which keeps every ACT op (Exp, Tanh, Relu) inside the single
`exp_and_others` activation table set: one LoadActFuncSet for the whole
kernel.  Likewise the SE-gate MLP sigmoids become 0.5+0.5*tanh(z/2).

Attention sub-tiles are packed 4x at PE quadrant offsets {0,32,64,96},
so exp/softmax-denominator/normalize/gate-matmul all run on 512-wide
free dims covering 2048 positions at a time.

Per-channel context sums (for the SE gate) ride tensor_scalar copies
with accum_out, split DVE/GPSIMD so no engine exceeds the PE wall;
the tiny [128] AllReduce across the 4 cores sharing each batch is
issued in two staggered halves so its latency hides under phase A.
"""

import sys

import numpy as np

if "/opt/trn_rl_repo" not in sys.path:
    sys.path.insert(0, "/opt/trn_rl_repo")

_B, _C, _D, _H, _W = 2, 64, 32, 64, 64
_K = 8
_NCORES = 8
_NPOS = (_B * _D // _NCORES) * _H * _W  # 32768 positions per core
_NT = 1024   # DMA tile width
_NS = 512    # attention sub-tile width (pack factor 4)
_NPAIR = _NPOS // (2 * _NT)  # 16 pairs of tiles (2048 positions each)

# phase-B combine: pair p -> direct-from-PSUM DVE path if p % _BCOMB_MOD == 0,
# else ACT copies psum->bf16 first so the DVE combine runs in 16-bit fast mode
_BCOMB_MOD = 3

_NC_CACHE = {}


def _build_nc(repeat=1, no_cc=False, fake_cc=False):
    import concourse.bass as bass  # noqa: F401
    import concourse.bacc as bacc
    import concourse.mybir as mybir
    from concourse import tile
    from concourse.alu_op_type import AluOpType

    f32 = mybir.dt.float32
    bf16 = mybir.dt.bfloat16
    AF = mybir.ActivationFunctionType

    nc = bacc.Bacc(None, num_devices=1 if no_cc else _NCORES)
    no_cc = no_cc or fake_cc

    xs_d = nc.declare_dram_parameter("xs", [128, _NPOS], bf16, isOutput=False)
    # params packed into two blocks: one DMA + one bf16 convert at start
    pbf_d = nc.declare_dram_parameter("pbf", [128, 456], f32, isOutput=False)
    pf_d = nc.declare_dram_parameter("pf32", [128, 209], f32, isOutput=False)
    out_d = nc.declare_dram_parameter("out", [128, _NPOS // 2], bf16,
                                      isOutput=True)

    cc_in = nc.dram_tensor("cc_in", [128, 1], f32)
    cc_out = nc.dram_tensor("cc_out", [128, 1], f32)
    cc_in2 = nc.dram_tensor("cc_in2", [128, 1], f32)
    cc_out2 = nc.dram_tensor("cc_out2", [128, 1], f32)

    ntiles = _NPOS // _NT  # 32

    rep_range = range(repeat)
    with tile.TileContext(nc) as tc:
        with (
            tc.tile_pool(name="const", bufs=1) as cpool,
            tc.tile_pool(name="res", bufs=1) as rpool,
            tc.tile_pool(name="work", bufs=4) as wpool,
        ):
            # param loads ride the idle ACT sequencer so the SP queue
            # head belongs to the input stream from cycle zero
            pbf_s = cpool.tile([128, 456], f32)
            nc.scalar.dma_start(pbf_s[:], pbf_d[:])
            pf_s = cpool.tile([128, 209], f32)
            nc.scalar.dma_start(pf_s[:], pf_d[:])
            pbf16 = cpool.tile([128, 456], bf16)
            nc.vector.tensor_copy(pbf16[:], pbf_s[:])
            # views into the packed blocks
            tokbf = pbf16[:, 0:32]
            b36bf = pbf16[:, 32:68]
            b4bf = pbf16[:, 68:72]
            bt36bf = pbf16[0:36, 72:200]
            g4abf = pbf16[:, 200:328]
            g4bbf = pbf16[:, 328:456]
            wst_s = pf_s[:, 0:16]
            wglf_s = pf_s[0:16, 16:80]
            wghf_s = pf_s[0:16, 80:144]
            i1_s = pf_s[:, 144:208]
            bgh_s = pf_s[:, 208:209]

            for _rep in rep_range:
                sxbf = rpool.tile([128, _NPOS], bf16)       # 64 KB/part
                gat = rpool.tile([128, _NPOS // 2], bf16)   # 32 KB/part
                rs_cols = rpool.tile([128, _NPAIR], f32)    # per-pair row sums
                junkD = rpool.tile([128, 2 * _NT], bf16)

                # ---- Phase A: stream x, attention weights + context sums ----
                # Software-pipelined over quads (2 pairs = 4096 positions):
                # iteration `it` emits, per engine stream,
                #   PE : scores_it | psRB_{it-1} | denom_it | psG_{it-2}
                #   ACT: exp_it | tanh_{it-2}
                #   DVE: rowsums_it | En_{it-1} | recip_it
                # so every op's cross-engine inputs are a full iteration old
                # and no engine waits mid-stream.
                NQ = _NPAIR // 2
                EQ = [None] * NQ     # per-quad [E_p0, E_p1]
                RQ = [None] * NQ     # per-quad R36
                ENQ = [None] * NQ    # per-quad [En_p0, En_p1]
                psa_ctx = tc.tile_pool(name="psA", bufs=1, space="PSUM")
                with psa_ctx as psa:
                    for it in range(NQ + 2):
                        if it < NQ:
                            q = it
                            Es = []
                            for m in range(2):
                                p = 2 * q + m
                                for h in range(2):
                                    t = 2 * p + h
                                    sl = slice(t * _NT, (t + 1) * _NT)
                                    nc.sync.dma_start(sxbf[:, sl],
                                                      xs_d[:, sl])
                                # context row-sum rides a bf16 copy with
                                # accum_out (fast DVE mode; Pool rejects the
                                # accum opcode in walrus codegen); one op per
                                # 2048-position pair amortizes the overhead
                                nc.vector.tensor_scalar(
                                    junkD[:], sxbf[:, 2 * p * _NT:
                                                   2 * (p + 1) * _NT],
                                    1.0, 0.0,
                                    AluOpType.mult, AluOpType.add,
                                    accum_out=rs_cols[:, p:p + 1],
                                )
                                psS = psa.tile([128, _NS], f32, tag="ps2k",
                                               name="psS", bufs=4)
                                for j in range(4):
                                    nc.tensor.matmul(
                                        psS[32 * j:32 * (j + 1), :], tokbf,
                                        sxbf[:, (4 * p + j) * _NS:
                                             (4 * p + j + 1) * _NS],
                                        start=True, stop=True,
                                        tile_position=(0, 32 * j),
                                    )
                                E = wpool.tile([128, _NS], bf16, tag="E",
                                               bufs=6)
                                nc.scalar.activation(E[:], psS[:], AF.Exp)
                                Es.append(E)
                            EQ[q] = Es
                        # PE: recip broadcast for quad it-1
                        q1 = it - 1
                        psRBs = []
                        if 0 <= q1 < NQ:
                            for m in range(2):
                                psRB = psa.tile([128, _NS], f32,
                                                tag="ps2k", name="psRB",
                                                bufs=4)
                                nc.tensor.matmul(
                                    psRB[:], bt36bf[32 * m:32 * m + 4, :],
                                    RQ[q1][32 * m:32 * m + 4, :],
                                    start=True, stop=True,
                                    tile_position=(32 * m, 0),
                                )
                                psRBs.append(psRB)
                        # PE: softmax denominators for quad it
                        psD = None
                        if it < NQ:
                            psD = psa.tile([36, _NS], f32, tag="ps2k",
                                           name="psD", bufs=4)
                            # first mm writes all 36 rows so the [36,512]
                            # recip never reads uninitialized PSUM
                            nc.tensor.matmul(psD[:], b36bf, EQ[it][0][:],
                                             start=True, stop=True,
                                             tile_position=(0, 0))
                            nc.tensor.matmul(psD[32:36, :], b4bf,
                                             EQ[it][1][:],
                                             start=True, stop=True,
                                             tile_position=(0, 32),
                                             skip_group_check=True)
                        # DVE: normalized attention for quad it-1
                        if 0 <= q1 < NQ:
                            Ens = []
                            for m in range(2):
                                En = wpool.tile([128, _NS], bf16, tag="En",
                                                bufs=6)
                                nc.vector.tensor_tensor(
                                    out=En[:], in0=EQ[q1][m][:],
                                    in1=psRBs[m][:], op=AluOpType.mult,
                                )
                                Ens.append(En)
                            ENQ[q1] = Ens
                            EQ[q1] = None
                        # DVE: reciprocal for quad it
                        if it < NQ:
                            R36 = wpool.tile([36, _NS], bf16, tag="R36",
                                             bufs=3)
                            with nc.allow_low_precision(
                                    "softmax recip in bf16: ~0.4% on attn "
                                    "weights, far under the 2e-2 gate"):
                                nc.vector.reciprocal(R36[:], psD[:])
                            RQ[it] = R36
                        # PE: gate matmuls + ACT: tanh cache for quad it-2
                        q2 = it - 2
                        if 0 <= q2 < NQ:
                            for m in range(2):
                                p = 2 * q2 + m
                                psG = psa.tile([128, 2 * _NS], f32,
                                               tag="psG", name="psG",
                                               bufs=2)
                                nc.tensor.matmul(psG[:, 0:_NS], g4abf,
                                                 ENQ[q2][m][:],
                                                 start=True, stop=True)
                                nc.tensor.matmul(psG[:, _NS:2 * _NS],
                                                 g4bbf, ENQ[q2][m][:],
                                                 start=True, stop=True)
                                # gate cache: tanh((psG + bg2)/2); sigmoid
                                # folds into phase B as (3 + tanh)/2
                                nc.scalar.activation(
                                    gat[:, _NT * p:_NT * (p + 1)], psG[:],
                                    AF.Tanh, bias=bgh_s, scale=0.5,
                                )
                                if p % 2 == 0:
                                    # in-place g3 = 3 + tanh so this pair's
                                    # phase-B combine is a fast 16-bit mult
                                    with nc.allow_low_precision(
                                            "bf16 gate cache, ~0.3%"):
                                        nc.vector.tensor_scalar(
                                            gat[:, _NT * p:_NT * (p + 1)],
                                            gat[:, _NT * p:_NT * (p + 1)],
                                            1.0, 3.0,
                                            AluOpType.mult, AluOpType.add,
                                        )
                            ENQ[q2] = None
                        if it == NQ // 2 - 1 and not no_cc:
                            # first-half context partial: its AllReduce
                            # latency hides under the second half of phase A
                            rs_a = rpool.tile([128, 1], f32)
                            nc.vector.tensor_reduce(
                                rs_a[:], rs_cols[:, 0:_NPAIR // 2],
                                axis=mybir.AxisListType.X, op=AluOpType.add,
                            )
                            nc.sync.dma_start(cc_in[:], rs_a[:])
                            nc.gpsimd.collective_compute(
                                "AllReduce", AluOpType.add,
                                replica_groups=[[0, 1, 2, 3], [4, 5, 6, 7]],
                                ins=[cc_in[:]], outs=[cc_out[:]],
                            )
                            cc_sb = rpool.tile([128, 2], f32)
                            nc.sync.dma_start(cc_sb[:, 0:1], cc_out[:])

                    # ---- second-half context AllReduce + gate MLP ----
                    rs = rpool.tile([128, 1], f32)
                    nc.vector.tensor_reduce(
                        rs[:], rs_cols[:, _NPAIR // 2:_NPAIR],
                        axis=mybir.AxisListType.X, op=AluOpType.add,
                    )
                    if no_cc:
                        ctxs = rs
                    else:
                        nc.sync.dma_start(cc_in2[:], rs[:])
                        nc.gpsimd.collective_compute(
                            "AllReduce", AluOpType.add,
                            replica_groups=[[0, 1, 2, 3], [4, 5, 6, 7]],
                            ins=[cc_in2[:]], outs=[cc_out2[:]],
                        )
                        nc.sync.dma_start(cc_sb[:, 1:2], cc_out2[:])
                        ctxs = rpool.tile([128, 1], f32)
                        nc.vector.tensor_reduce(
                            ctxs[:], cc_sb[:], axis=mybir.AxisListType.X,
                            op=AluOpType.add,
                        )
                    ps1 = psa.tile([16, 1], f32, tag="ps2k", name="ps1", bufs=4)
                    nc.tensor.matmul(ps1[:], wst_s, ctxs[:],
                                     start=True, stop=True)
                    sh = rpool.tile([16, 1], f32)
                    nc.scalar.activation(sh[:], ps1[:], AF.Relu)
                    ps2 = psa.tile([64, 1], f32, tag="ps2k", name="ps2", bufs=4)
                    nc.tensor.matmul(ps2[:], wglf_s, sh[:],
                                     start=True, stop=True)
                    ps3 = psa.tile([64, 1], f32, tag="ps2k", name="ps3", bufs=4)
                    nc.tensor.matmul(ps3[:], wghf_s, sh[:],
                                     start=True, stop=True)
                    # w = 2*sigmoid(z); base/2 uses sigmoid(z) = .5+.5*tanh(z/2)
                    wvt = rpool.tile([128, 1], f32)
                    nc.scalar.activation(wvt[0:64, :], ps2[:], AF.Tanh,
                                         scale=0.5)
                    nc.scalar.activation(wvt[64:128, :], ps3[:], AF.Tanh,
                                         scale=0.5)
                    # wsel = I1h*(1+t) = I1*(0.5+0.5*t) = I1*sigmoid(z)
                    wsel = rpool.tile([128, 64], bf16)
                    nc.vector.scalar_tensor_tensor(
                        wsel[:], i1_s, wvt[:, 0:1], i1_s,
                        AluOpType.mult, AluOpType.add,
                    )
                # ---- Phase B: base matmul + combine, stream out ----
                with (
                    tc.tile_pool(name="psB", bufs=4, space="PSUM") as psbp,
                    tc.tile_pool(name="outp", bufs=6) as opool,
                ):
                    for p in range(_NPAIR):
                        psB = psbp.tile([128, 2 * _NS], f32, tag="psB")
                        for b in range(2):
                            for beta in range(2):
                                v = 4 * p + 2 * b + beta
                                nc.tensor.matmul(
                                    psB[64 * beta:64 * beta + 64,
                                        b * _NS:(b + 1) * _NS],
                                    wsel[:], sxbf[:, v * _NS:(v + 1) * _NS],
                                    start=True, stop=True,
                                )
                        outt = opool.tile([128, _NT], bf16, tag="outt")
                        gsl = gat[:, _NT * p:_NT * (p + 1)]
                        with nc.allow_low_precision(
                                "bf16 output: ~0.2% rounding, far under "
                                "the 2e-2 gate"):
                            if p % 2 == 0:
                                # g3 cached: ACT copies psum->bf16, then the
                                # combine is a 16-bit tensor_tensor mult
                                pb = opool.tile([128, _NT], bf16, tag="pb")
                                nc.scalar.activation(pb[:], psB[:], AF.Copy)
                                eng = nc.gpsimd if p in (0, 2, 4) else nc.vector
                                eng.tensor_tensor(
                                    out=outt[:], in0=gsl, in1=pb[:],
                                    op=AluOpType.mult,
                                )
                            else:
                                nc.vector.scalar_tensor_tensor(
                                    outt[:], gsl, 3.0, psB[:],
                                    AluOpType.add, AluOpType.mult,
                                )
                        nc.sync.dma_start(
                            out_d[:, _NT * p:_NT * (p + 1)],
                            outt[:],
                        )

    nc.compile()
    nc.finalize()
    return nc


def _get_nc(repeat=1, no_cc=False, fake_cc=False):
    key = f"nc{repeat}_{no_cc}_{fake_cc}"
    if key not in _NC_CACHE:
        _NC_CACHE[key] = _build_nc(repeat, no_cc, fake_cc)
    return _NC_CACHE[key]


def _host_params(inputs):
    f = np.float32
    tokens = np.asarray(inputs["tokens"], f)
    scale = float(np.asarray(inputs["scale"]).reshape(-1)[0])
    sf = _C ** -0.5
    tok32 = np.zeros((128, 32), f)
    tok32[0:64, 0:_K] = tokens.T * sf
    tok32[64:128, 0:_K] = tokens.T * sf
    tok_t = tokens @ np.asarray(inputs["W_t2f"], f).T + np.asarray(
        inputs["b_t2f"], f)
    M = (tok_t @ np.asarray(inputs["W_delta"], f).T) * scale
    W_gate = np.asarray(inputs["W_gate"], f)
    G = M @ W_gate.T  # [8, 64]
    bg2v = (W_gate @ (np.asarray(inputs["b_delta"], f) * scale)
            + np.asarray(inputs["b_gate"], f))
    # tanh bias: (psG + bg2)/2, stacked for both row halves
    bgh = 0.5 * np.concatenate([bg2v, bg2v])[:, None]  # [128,1]
    # quadrant-packed selector / replication matrices (bands at 32j)
    B4 = np.zeros((128, 4), f)
    B36 = np.zeros((128, 36), f)
    Bt36 = np.zeros((36, 128), f)
    G4a = np.zeros((128, 128), f)
    G4b = np.zeros((128, 128), f)
    for j in range(4):
        B4[32 * j:32 * j + 8, j] = 1.0
        B36[32 * j:32 * j + 8, j] = 1.0
    # rows 4-35 of the pair-0 denominator tile get E[0,:] (finite, unused)
    B36[0, 4:36] = 1.0
    for m in range(2):
        for j in range(4):
            Bt36[32 * m + j, 32 * j:32 * j + 8] = 1.0
    for k in range(_K):
        G4a[k, 0:64] = G[k]
        G4a[32 + k, 64:128] = G[k]
        G4b[64 + k, 0:64] = G[k]
        G4b[96 + k, 64:128] = G[k]
    WsT = np.ascontiguousarray(
        np.asarray(inputs["W_shared"], f).T / (_D * _H * _W))
    WglfT = np.ascontiguousarray(np.asarray(inputs["W_glf"], f).T)
    WghfT = np.ascontiguousarray(np.asarray(inputs["W_ghf"], f).T)
    eye1 = 0.5 * np.eye(64, dtype=f)
    I1 = np.ascontiguousarray(np.concatenate([eye1, eye1], 0))
    pbf = np.zeros((128, 456), f)
    pbf[:, 0:32] = tok32
    pbf[:, 32:68] = B36
    pbf[:, 68:72] = B4
    pbf[0:36, 72:200] = Bt36
    pbf[:, 200:328] = G4a
    pbf[:, 328:456] = G4b
    pf32 = np.zeros((128, 209), f)
    pf32[:, 0:16] = WsT
    pf32[0:16, 16:80] = WglfT
    pf32[0:16, 80:144] = WghfT
    pf32[:, 144:208] = I1
    pf32[:, 208:209] = bgh
    return {"pbf": pbf, "pf32": pf32}


def _make_in_maps(inputs):
    import ml_dtypes

    x_hf = np.asarray(inputs["x_hf"], np.float32)
    x_lf = np.asarray(inputs["x_lf"], np.float32)
    params = _host_params(inputs)
    in_maps = []
    for i in range(_NCORES):
        b, d0 = i // 4, 8 * (i % 4)
        xl = x_lf[b, :, d0:d0 + 8].reshape(64, -1)
        xh = x_hf[b, :, d0:d0 + 8].reshape(64, -1)
        xs = np.ascontiguousarray(
            np.concatenate([xl, xh], 0)).astype(ml_dtypes.bfloat16)
        m = {"xs": xs}
        m.update(params)
        in_maps.append(m)
    return in_maps


def _decode_out(raw):
    # raw [128, NPOS/2] bf16: row 64*beta+c, col 512*(2p+b)+f
    #  -> channel c, position 512*(4p+2b+beta)+f
    o = np.asarray(raw).astype(np.float32)
    o = o.reshape(2, 64, _NPAIR, 2, _NS)          # [beta, c, p, b, f]
    o = o.transpose(1, 2, 3, 0, 4)                # [c, p, b, beta, f]
    return o.reshape(64, _NPOS)


def kernel(**inputs):
    from concourse.bass_utils import run_bass_kernel_spmd

    in_maps = _make_in_maps(inputs)
    nc = _get_nc()
    res = run_bass_kernel_spmd(nc, in_maps, list(range(_NCORES)))
    out = np.empty((_B, _C, _D, _H, _W), np.float32)
    for i in range(_NCORES):
        b, d0 = i // 4, 8 * (i % 4)
        out[b, :, d0:d0 + 8] = _decode_out(res.results[i]["out"]).reshape(
            64, 8, _H, _W)
    return out


# revision 27
# speedup vs baseline: 8.3631x; 1.5426x over previous
"""Adaptive frequency reassemble kernel for 8 TRN2 NeuronCores.

Sharding: pure data parallel over (B, D): core i owns batch b=i//4 and
d-slab [8*(i%4), 8*(i%4)+8) -> 32768 positions/core.  x_lf / x_hf are
stacked into one [128, 32768] tensor per core (lf channels on partitions
0-63, hf on 64-127) and fed to the device in bf16, halving the input
HBM traffic vs f32.  The output leaves the device in bf16 on all 128
partitions ([128, NPOS/2] with channels x position-parity packing),
quartering the output DMA per-partition byte count vs the f32 [64, NPOS]
layout.  Host casts both ways (cheap, not on the device critical path).

Algebraic folds (host, exact):
  tok_t  = tokens @ W_t2f.T + b_t2f
  M      = (tok_t @ W_delta.T) * scale
  G      = M @ W_gate.T                  [8, 64]
  bg2    = W_gate @ (b_delta*scale) + b_gate
so   gate_pre = G.T @ softmax_weights + bg2, and the final combine uses
  out = base*(1+sigmoid(gate_pre)) = (base/2) * (3 + tanh(gate_pre/2)),
which keeps every ACT op (Exp, Tanh, Relu) inside the single
`exp_and_others` activation table set: one LoadActFuncSet for the whole
kernel.  Likewise the SE-gate MLP sigmoids become 0.5+0.5*tanh(z/2).

Attention sub-tiles are packed 4x at PE quadrant offsets {0,32,64,96},
so exp/softmax-denominator/normalize/gate-matmul all run on 512-wide
free dims covering 2048 positions at a time.

Per-channel context sums (for the SE gate) ride tensor_scalar copies
with accum_out, split DVE/GPSIMD so no engine exceeds the PE wall;
the tiny [128] AllReduce across the 4 cores sharing each batch is
issued in two staggered halves so its latency hides under phase A.
"""

import sys

import numpy as np

if "/opt/trn_rl_repo" not in sys.path:
    sys.path.insert(0, "/opt/trn_rl_repo")

_B, _C, _D, _H, _W = 2, 64, 32, 64, 64
_K = 8
_NCORES = 8
_NPOS = (_B * _D // _NCORES) * _H * _W  # 32768 positions per core
_NT = 1024   # DMA tile width
_NS = 512    # attention sub-tile width (pack factor 4)
_NPAIR = _NPOS // (2 * _NT)  # 16 pairs of tiles (2048 positions each)

# phase-B combine: pair p -> direct-from-PSUM DVE path if p % _BCOMB_MOD == 0,
# else ACT copies psum->bf16 first so the DVE combine runs in 16-bit fast mode
_BCOMB_MOD = 3

_NC_CACHE = {}


def _build_nc(repeat=1, no_cc=False, fake_cc=False):
    import concourse.bass as bass  # noqa: F401
    import concourse.bacc as bacc
    import concourse.mybir as mybir
    from concourse import tile
    from concourse.alu_op_type import AluOpType

    f32 = mybir.dt.float32
    bf16 = mybir.dt.bfloat16
    AF = mybir.ActivationFunctionType

    nc = bacc.Bacc(None, num_devices=1 if no_cc else _NCORES)
    no_cc = no_cc or fake_cc

    xs_d = nc.declare_dram_parameter("xs", [128, _NPOS], bf16, isOutput=False)
    # params packed into two blocks: one DMA + one bf16 convert at start
    pbf_d = nc.declare_dram_parameter("pbf", [128, 456], f32, isOutput=False)
    pf_d = nc.declare_dram_parameter("pf32", [128, 209], f32, isOutput=False)
    out_d = nc.declare_dram_parameter("out", [128, _NPOS // 2], bf16,
                                      isOutput=True)

    cc_in = nc.dram_tensor("cc_in", [128, 1], f32)
    cc_out = nc.dram_tensor("cc_out", [128, 1], f32)
    cc_in2 = nc.dram_tensor("cc_in2", [128, 1], f32)
    cc_out2 = nc.dram_tensor("cc_out2", [128, 1], f32)

    ntiles = _NPOS // _NT  # 32

    rep_range = range(repeat)
    with tile.TileContext(nc) as tc:
        with (
            tc.tile_pool(name="const", bufs=1) as cpool,
            tc.tile_pool(name="res", bufs=1) as rpool,
            tc.tile_pool(name="work", bufs=4) as wpool,
        ):
            # param loads ride the idle ACT sequencer so the SP queue
            # head belongs to the input stream from cycle zero
            pbf_s = cpool.tile([128, 456], f32)
            nc.scalar.dma_start(pbf_s[:], pbf_d[:])
            pf_s = cpool.tile([128, 209], f32)
            nc.scalar.dma_start(pf_s[:], pf_d[:])
            pbf16 = cpool.tile([128, 456], bf16)
            # token block first: the first scores matmul only needs cols 0:32
            nc.vector.tensor_copy(pbf16[:, 0:32], pbf_s[:, 0:32])
            nc.vector.tensor_copy(pbf16[:, 32:456], pbf_s[:, 32:456])
            # views into the packed blocks
            tokbf = pbf16[:, 0:32]
            b36bf = pbf16[:, 32:68]
            b4bf = pbf16[:, 68:72]
            bt36bf = pbf16[0:36, 72:200]
            g4abf = pbf16[:, 200:328]
            g4bbf = pbf16[:, 328:456]
            wst_s = pf_s[:, 0:16]
            wglf_s = pf_s[0:16, 16:80]
            wghf_s = pf_s[0:16, 80:144]
            i1_s = pf_s[:, 144:208]
            bgh_s = pf_s[:, 208:209]

            for _rep in rep_range:
                sxbf = rpool.tile([128, _NPOS], bf16)       # 64 KB/part
                gat = rpool.tile([128, _NPOS // 2], bf16)   # 32 KB/part
                rs_cols = rpool.tile([128, _NPAIR], f32)    # per-pair row sums
                junkD = rpool.tile([128, 2 * _NT], bf16)

                # ---- Phase A: stream x, attention weights + context sums ----
                # Software-pipelined over quads (2 pairs = 4096 positions):
                # iteration `it` emits, per engine stream,
                #   PE : scores_it | psRB_{it-1} | denom_it | psG_{it-2}
                #   ACT: exp_it | tanh_{it-2}
                #   DVE: rowsums_it | En_{it-1} | recip_it
                # so every op's cross-engine inputs are a full iteration old
                # and no engine waits mid-stream.
                NQ = _NPAIR // 2
                EQ = [None] * NQ     # per-quad [E_p0, E_p1]
                RQ = [None] * NQ     # per-quad R36
                ENQ = [None] * NQ    # per-quad [En_p0, En_p1]
                psa_ctx = tc.tile_pool(name="psA", bufs=1, space="PSUM")
                with psa_ctx as psa:
                    for it in range(NQ + 2):
                        if it == NQ:
                            # second-half context AllReduce + gate MLP,
                            # emitted before the drain so wsel resolves
                            # while the attention tail is still running
                            rs = rpool.tile([128, 1], f32)
                            nc.vector.tensor_reduce(
                                rs[:], rs_cols[:, _NPAIR // 2:_NPAIR],
                                axis=mybir.AxisListType.X, op=AluOpType.add,
                            )
                            if no_cc:
                                ctxs = rs
                            else:
                                nc.sync.dma_start(cc_in2[:], rs[:])
                                nc.gpsimd.collective_compute(
                                    "AllReduce", AluOpType.add,
                                    replica_groups=[[0, 1, 2, 3], [4, 5, 6, 7]],
                                    ins=[cc_in2[:]], outs=[cc_out2[:]],
                                )
                                nc.sync.dma_start(cc_sb[:, 1:2], cc_out2[:])
                                ctxs = rpool.tile([128, 1], f32)
                                nc.vector.tensor_reduce(
                                    ctxs[:], cc_sb[:], axis=mybir.AxisListType.X,
                                    op=AluOpType.add,
                                )
                            ps1 = psa.tile([16, 1], f32, tag="ps2k", name="ps1", bufs=4)
                            nc.tensor.matmul(ps1[:], wst_s, ctxs[:],
                                             start=True, stop=True)
                            sh = rpool.tile([16, 1], f32)
                            nc.scalar.activation(sh[:], ps1[:], AF.Relu)
                            ps2 = psa.tile([64, 1], f32, tag="ps2k", name="ps2", bufs=4)
                            nc.tensor.matmul(ps2[:], wglf_s, sh[:],
                                             start=True, stop=True)
                            ps3 = psa.tile([64, 1], f32, tag="ps2k", name="ps3", bufs=4)
                            nc.tensor.matmul(ps3[:], wghf_s, sh[:],
                                             start=True, stop=True)
                            # w = 2*sigmoid(z); base/2 uses sigmoid(z) = .5+.5*tanh(z/2)
                            wvt = rpool.tile([128, 1], f32)
                            nc.scalar.activation(wvt[0:64, :], ps2[:], AF.Tanh,
                                                 scale=0.5)
                            nc.scalar.activation(wvt[64:128, :], ps3[:], AF.Tanh,
                                                 scale=0.5)
                            # wsel = I1h*(1+t) = I1*(0.5+0.5*t) = I1*sigmoid(z)
                            wsel = rpool.tile([128, 64], bf16)
                            nc.vector.scalar_tensor_tensor(
                                wsel[:], i1_s, wvt[:, 0:1], i1_s,
                                AluOpType.mult, AluOpType.add,
                            )
                        if it < NQ:
                            q = it
                            Es = []
                            for m in range(2):
                                p = 2 * q + m
                                for h in range(2):
                                    t = 2 * p + h
                                    sl = slice(t * _NT, (t + 1) * _NT)
                                    nc.sync.dma_start(sxbf[:, sl],
                                                      xs_d[:, sl])
                                # context row-sum rides a bf16 copy with
                                # accum_out (fast DVE mode; Pool rejects the
                                # accum opcode in walrus codegen); one op per
                                # 2048-position pair amortizes the overhead
                                nc.vector.tensor_scalar(
                                    junkD[:], sxbf[:, 2 * p * _NT:
                                                   2 * (p + 1) * _NT],
                                    1.0, 0.0,
                                    AluOpType.mult, AluOpType.add,
                                    accum_out=rs_cols[:, p:p + 1],
                                )
                                psS = psa.tile([128, _NS], f32, tag="ps2k",
                                               name="psS", bufs=4)
                                for j in range(4):
                                    nc.tensor.matmul(
                                        psS[32 * j:32 * (j + 1), :], tokbf,
                                        sxbf[:, (4 * p + j) * _NS:
                                             (4 * p + j + 1) * _NS],
                                        start=True, stop=True,
                                        tile_position=(0, 32 * j),
                                    )
                                E = wpool.tile([128, _NS], bf16, tag="E",
                                               bufs=6)
                                nc.scalar.activation(E[:], psS[:], AF.Exp)
                                Es.append(E)
                            EQ[q] = Es
                        # PE: recip broadcast for quad it-1
                        q1 = it - 1
                        psRBs = []
                        if 0 <= q1 < NQ:
                            for m in range(2):
                                psRB = psa.tile([128, _NS], f32,
                                                tag="ps2k", name="psRB",
                                                bufs=4)
                                nc.tensor.matmul(
                                    psRB[:], bt36bf[32 * m:32 * m + 4, :],
                                    RQ[q1][32 * m:32 * m + 4, :],
                                    start=True, stop=True,
                                    tile_position=(32 * m, 0),
                                )
                                psRBs.append(psRB)
                        # PE: softmax denominators for quad it
                        psD = None
                        if it < NQ:
                            psD = psa.tile([36, _NS], f32, tag="ps2k",
                                           name="psD", bufs=4)
                            # first mm writes all 36 rows so the [36,512]
                            # recip never reads uninitialized PSUM
                            nc.tensor.matmul(psD[:], b36bf, EQ[it][0][:],
                                             start=True, stop=True,
                                             tile_position=(0, 0))
                            nc.tensor.matmul(psD[32:36, :], b4bf,
                                             EQ[it][1][:],
                                             start=True, stop=True,
                                             tile_position=(0, 32),
                                             skip_group_check=True)
                        # DVE: normalized attention for quad it-1
                        if 0 <= q1 < NQ:
                            Ens = []
                            for m in range(2):
                                En = wpool.tile([128, _NS], bf16, tag="En",
                                                bufs=6)
                                nc.vector.tensor_tensor(
                                    out=En[:], in0=EQ[q1][m][:],
                                    in1=psRBs[m][:], op=AluOpType.mult,
                                )
                                Ens.append(En)
                            ENQ[q1] = Ens
                            EQ[q1] = None
                        # DVE: reciprocal for quad it
                        if it < NQ:
                            R36 = wpool.tile([36, _NS], bf16, tag="R36",
                                             bufs=3)
                            with nc.allow_low_precision(
                                    "softmax recip in bf16: ~0.4% on attn "
                                    "weights, far under the 2e-2 gate"):
                                nc.vector.reciprocal(R36[:], psD[:])
                            RQ[it] = R36
                        # PE: gate matmuls + ACT: tanh cache for quad it-2
                        q2 = it - 2
                        if 0 <= q2 < NQ:
                            for m in range(2):
                                p = 2 * q2 + m
                                psG = psa.tile([128, 2 * _NS], f32,
                                               tag="psG", name="psG",
                                               bufs=2)
                                nc.tensor.matmul(psG[:, 0:_NS], g4abf,
                                                 ENQ[q2][m][:],
                                                 start=True, stop=True)
                                nc.tensor.matmul(psG[:, _NS:2 * _NS],
                                                 g4bbf, ENQ[q2][m][:],
                                                 start=True, stop=True)
                                # gate cache: tanh((psG + bg2)/2); sigmoid
                                # folds into phase B as (3 + tanh)/2
                                nc.scalar.activation(
                                    gat[:, _NT * p:_NT * (p + 1)], psG[:],
                                    AF.Tanh, bias=bgh_s, scale=0.5,
                                )
                                if p % 2 == 0:
                                    # in-place g3 = 3 + tanh so this pair's
                                    # phase-B combine is a fast 16-bit mult
                                    with nc.allow_low_precision(
                                            "bf16 gate cache, ~0.3%"):
                                        nc.vector.tensor_scalar(
                                            gat[:, _NT * p:_NT * (p + 1)],
                                            gat[:, _NT * p:_NT * (p + 1)],
                                            1.0, 3.0,
                                            AluOpType.mult, AluOpType.add,
                                        )
                            ENQ[q2] = None
                        if it == NQ // 2 - 1 and not no_cc:
                            # first-half context partial: its AllReduce
                            # latency hides under the second half of phase A
                            rs_a = rpool.tile([128, 1], f32)
                            nc.vector.tensor_reduce(
                                rs_a[:], rs_cols[:, 0:_NPAIR // 2],
                                axis=mybir.AxisListType.X, op=AluOpType.add,
                            )
                            nc.sync.dma_start(cc_in[:], rs_a[:])
                            nc.gpsimd.collective_compute(
                                "AllReduce", AluOpType.add,
                                replica_groups=[[0, 1, 2, 3], [4, 5, 6, 7]],
                                ins=[cc_in[:]], outs=[cc_out[:]],
                            )
                            cc_sb = rpool.tile([128, 2], f32)
                            nc.sync.dma_start(cc_sb[:, 0:1], cc_out[:])

                    # ---- Phase B: base matmul + combine, stream out ----
                    # psB halves reuse the phase-A "ps2k" psum tag, so the
                    # first base matmuls overlap the attention drain instead
                    # of waiting for a pool transition
                    with tc.tile_pool(name="outp", bufs=6) as opool:
                        for p in range(_NPAIR):
                            outt = opool.tile([128, _NT], bf16, tag="outt")
                            halves = []
                            for b in range(2):
                                psBh = psa.tile([128, _NS], f32, tag="ps2k",
                                                name="psB", bufs=4)
                                for beta in range(2):
                                    v = 4 * p + 2 * b + beta
                                    nc.tensor.matmul(
                                        psBh[64 * beta:64 * beta + 64, :],
                                        wsel[:],
                                        sxbf[:, v * _NS:(v + 1) * _NS],
                                        start=True, stop=True,
                                    )
                                halves.append(psBh)
                            with nc.allow_low_precision(
                                    "bf16 output: ~0.2% rounding, far "
                                    "under the 2e-2 gate"):
                                for b in range(2):
                                    osl = outt[:, b * _NS:(b + 1) * _NS]
                                    gsl = gat[:, _NT * p + b * _NS:
                                              _NT * p + (b + 1) * _NS]
                                    if p % 2 == 0:
                                        # g3 cached: ACT copies psum->bf16,
                                        # combine is a 16-bit mult
                                        pb = opool.tile([128, _NS], bf16,
                                                        tag="pb")
                                        nc.scalar.activation(
                                            pb[:], halves[b][:], AF.Copy)
                                        eng = (nc.gpsimd if p in (0, 2, 4)
                                               else nc.vector)
                                        eng.tensor_tensor(
                                            out=osl, in0=gsl, in1=pb[:],
                                            op=AluOpType.mult,
                                        )
                                    else:
                                        nc.vector.scalar_tensor_tensor(
                                            osl, gsl, 3.0, halves[b][:],
                                            AluOpType.add, AluOpType.mult,
                                        )
                            nc.sync.dma_start(
                                out_d[:, _NT * p:_NT * (p + 1)],
                                outt[:],
                            )

    nc.compile()
    nc.finalize()
    return nc


def _get_nc(repeat=1, no_cc=False, fake_cc=False):
    key = f"nc{repeat}_{no_cc}_{fake_cc}"
    if key not in _NC_CACHE:
        _NC_CACHE[key] = _build_nc(repeat, no_cc, fake_cc)
    return _NC_CACHE[key]


def _host_params(inputs):
    f = np.float32
    tokens = np.asarray(inputs["tokens"], f)
    scale = float(np.asarray(inputs["scale"]).reshape(-1)[0])
    sf = _C ** -0.5
    tok32 = np.zeros((128, 32), f)
    tok32[0:64, 0:_K] = tokens.T * sf
    tok32[64:128, 0:_K] = tokens.T * sf
    tok_t = tokens @ np.asarray(inputs["W_t2f"], f).T + np.asarray(
        inputs["b_t2f"], f)
    M = (tok_t @ np.asarray(inputs["W_delta"], f).T) * scale
    W_gate = np.asarray(inputs["W_gate"], f)
    G = M @ W_gate.T  # [8, 64]
    bg2v = (W_gate @ (np.asarray(inputs["b_delta"], f) * scale)
            + np.asarray(inputs["b_gate"], f))
    # tanh bias: (psG + bg2)/2, stacked for both row halves
    bgh = 0.5 * np.concatenate([bg2v, bg2v])[:, None]  # [128,1]
    # quadrant-packed selector / replication matrices (bands at 32j)
    B4 = np.zeros((128, 4), f)
    B36 = np.zeros((128, 36), f)
    Bt36 = np.zeros((36, 128), f)
    G4a = np.zeros((128, 128), f)
    G4b = np.zeros((128, 128), f)
    for j in range(4):
        B4[32 * j:32 * j + 8, j] = 1.0
        B36[32 * j:32 * j + 8, j] = 1.0
    # rows 4-35 of the pair-0 denominator tile get E[0,:] (finite, unused)
    B36[0, 4:36] = 1.0
    for m in range(2):
        for j in range(4):
            Bt36[32 * m + j, 32 * j:32 * j + 8] = 1.0
    for k in range(_K):
        G4a[k, 0:64] = G[k]
        G4a[32 + k, 64:128] = G[k]
        G4b[64 + k, 0:64] = G[k]
        G4b[96 + k, 64:128] = G[k]
    WsT = np.ascontiguousarray(
        np.asarray(inputs["W_shared"], f).T / (_D * _H * _W))
    WglfT = np.ascontiguousarray(np.asarray(inputs["W_glf"], f).T)
    WghfT = np.ascontiguousarray(np.asarray(inputs["W_ghf"], f).T)
    eye1 = 0.5 * np.eye(64, dtype=f)
    I1 = np.ascontiguousarray(np.concatenate([eye1, eye1], 0))
    pbf = np.zeros((128, 456), f)
    pbf[:, 0:32] = tok32
    pbf[:, 32:68] = B36
    pbf[:, 68:72] = B4
    pbf[0:36, 72:200] = Bt36
    pbf[:, 200:328] = G4a
    pbf[:, 328:456] = G4b
    pf32 = np.zeros((128, 209), f)
    pf32[:, 0:16] = WsT
    pf32[0:16, 16:80] = WglfT
    pf32[0:16, 80:144] = WghfT
    pf32[:, 144:208] = I1
    pf32[:, 208:209] = bgh
    return {"pbf": pbf, "pf32": pf32}


def _make_in_maps(inputs):
    import ml_dtypes

    x_hf = np.asarray(inputs["x_hf"], np.float32)
    x_lf = np.asarray(inputs["x_lf"], np.float32)
    params = _host_params(inputs)
    in_maps = []
    for i in range(_NCORES):
        b, d0 = i // 4, 8 * (i % 4)
        xl = x_lf[b, :, d0:d0 + 8].reshape(64, -1)
        xh = x_hf[b, :, d0:d0 + 8].reshape(64, -1)
        xs = np.ascontiguousarray(
            np.concatenate([xl, xh], 0)).astype(ml_dtypes.bfloat16)
        m = {"xs": xs}
        m.update(params)
        in_maps.append(m)
    return in_maps


def _decode_out(raw):
    # raw [128, NPOS/2] bf16: row 64*beta+c, col 512*(2p+b)+f
    #  -> channel c, position 512*(4p+2b+beta)+f
    o = np.asarray(raw).astype(np.float32)
    o = o.reshape(2, 64, _NPAIR, 2, _NS)          # [beta, c, p, b, f]
    o = o.transpose(1, 2, 3, 0, 4)                # [c, p, b, beta, f]
    return o.reshape(64, _NPOS)


def kernel(**inputs):
    from concourse.bass_utils import run_bass_kernel_spmd

    in_maps = _make_in_maps(inputs)
    nc = _get_nc()
    res = run_bass_kernel_spmd(nc, in_maps, list(range(_NCORES)))
    out = np.empty((_B, _C, _D, _H, _W), np.float32)
    for i in range(_NCORES):
        b, d0 = i // 4, 8 * (i % 4)
        out[b, :, d0:d0 + 8] = _decode_out(res.results[i]["out"]).reshape(
            64, 8, _H, _W)
    return out
